# revision 9
# baseline (speedup 1.0000x reference)
"""Trainium2 Bass kernel for nn_NSELoss (segment-reduce NSE loss).

Contract: kernel(**inputs) takes the FULL inputs
  y_pred [16777216] f32, y_true [16777216] f32,
  stations [16777216] i32, station_std [1024] f32
and returns the full scalar output (f32), matching reference():
  err2 = (y_pred - y_true)^2
  sse/cnt = segment_sum(err2 / 1, stations, 1024)
  loss = mean over present stations of (sse/cnt) / (std+eps)^2

Sharding: data-parallel over N across 8 NeuronCores. Each core computes
partial sse[1024] and cnt[1024] on-device; host sums the 8 partial pairs
(the all-reduce of two [S] vectors) and finishes the normalization.

Device algorithm (per core, 2,097,152 elements as [128, 16384]):
station s = hi*32 + lo. For each 128-element column (one element per
partition), build bf16 one-hot masks on the Vector engine:
  B3[p, 0:32]  = (lo[p] == j)                       (stationary, exactly 0/1)
  AA[p, 0:32]  = (hi[p] == j) * err2_bf16[p]
  AA[p, 32:64] = (hi[p] == j)
and accumulate on the Tensor engine into PSUM across all 16384 columns:
  psum[32, 64] += B3^T @ AA
so psum[lo, hi] = sse and psum[lo, 32+hi] = cnt. One-hots are exact in
bf16 and PSUM accumulates in fp32, so cnt is exact; err2 is rounded to
bf16 (final loss rel err ~3e-6).

vs the previous 64x16 split: the 32-col stationary halves the LDWEIGHTS
row count, is_weight_onezero=True enables the binary-weight load path,
DVE mask builds put the broadcast operand in slot 0 (faster perf-mode),
and per-matmul semaphore increments are coalesced to one per block (with
all wait values renumbered) to keep 16k EVT_SEM writes off the event bus.
"""
import numpy as np
import ml_dtypes

import bass_rust
import concourse.bass as bass
import concourse.mybir as mybir
from concourse import tile as tile_mod
from concourse.tile import TileContext
from concourse.vector_clock import ScopedClock

F32 = mybir.dt.float32
BF16 = mybir.dt.bfloat16
I32 = mybir.dt.int32

N = 16_777_216
S = 1024
EPS = 1e-6
N_CORES = 8
P = 128
F = N // N_CORES // P          # 16384 free-dim elements per partition
L = 32                         # lo width (stationary one-hot)
H = 32                         # hi width;  s = hi*L + lo


# --- workarounds for this toolchain's walrus: it rejects >1 sync wait per
# --- instruction (setupSyncWait), including on Drain/NoOp (TPB_CTRL).

def _drain_and_barrier(self, tick_clock, wait_clock):
    nop0 = self.nc.sync.nop(nofuse=True)
    wait_clock.add_sem_waits(nop0.ins, ScopedClock({None: tick_clock.global_clock}))
    si = nop0.ins.sync_info
    waits = list(si.on_wait) if si is not None else []
    if len(waits) > 1:
        nop0.ins.sync_info = bass_rust.SyncInfo(on_wait=[waits[0]], on_update=[])
        for w in waits[1:]:
            nop = self.nc.sync.nop(nofuse=True)
            nop.ins.sync_info = bass_rust.SyncInfo(on_wait=[w], on_update=[])
    self.nc.sync.drain()
    self.nc.all_engine_barrier()
    popped = self.nc._tile_sem_poison_stack.pop()
    assert popped is self._sem_poison
    self.nc.clear_and_free_semaphores(list(self.sems.allocated().values()))
    self.nc.all_engine_barrier()


tile_mod.TileContext._drain_and_barrier = _drain_and_barrier


def _split_multi_waits(nc, max_waits=1):
    n = 0
    for f in nc.m.functions:
        for b in f.blocks:
            out, changed = [], False
            for i in b.instructions:
                si = i.sync_info
                waits = list(si.on_wait) if si is not None else []
                if len(waits) > max_waits:
                    for w in waits[:-max_waits]:
                        nop = bass_rust.InstNoOp(name=f"W-split-{n}")
                        n += 1
                        nop.engine = i.engine
                        nop.sync_info = bass_rust.SyncInfo(on_wait=[w], on_update=[])
                        out.append(nop)
                    i.sync_info = bass_rust.SyncInfo(
                        on_wait=waits[-max_waits:], on_update=list(si.on_update))
                    changed = True
                out.append(i)
            if changed:
                b.instructions = out


def _bulk_matmul_incs(nc):
    """Drop the per-matmul sem increments (keeping one per consecutive run)
    and remap every wait on that semaphore to the new counting.

    Every InstMatmult gets a then-inc(PE sem, 1) from the Tile auto-sync; at
    16k matmuls the EVT_SEM writes contend on the event bus. Walrus requires
    update_value == 1, so instead of bulk-incrementing we renumber: the PE
    sem counts matmul-RUN completions. A wait for original count X becomes a
    wait for the first kept update at-or-after X — released at that run's
    end, which is when the recycled mask buffer's last reader is done.
    """
    sem_id = None
    for f in nc.m.functions:
        for b in f.blocks:
            for i in b.instructions:
                if type(i).__name__ == "InstMatmult":
                    si = i.sync_info
                    ups = list(si.on_update) if si is not None else []
                    if ups and ups[0].update_mode == "sem-inc":
                        sem_id = ups[0].id
                        break
            if sem_id is not None:
                break
        if sem_id is not None:
            break
    if sem_id is None:
        return

    def upd_on(i):
        si = i.sync_info
        ups = list(si.on_update) if si is not None else []
        return [u for u in ups if u.id == sem_id and u.update_mode == "sem-inc"]

    orig_cum = 0
    kept = []                 # orig cumulative counts at kept updates
    drops = []
    all_insts = []
    for f in nc.m.functions:
        for b in f.blocks:
            run = []
            def flush():
                nonlocal orig_cum
                for j, mi in enumerate(run):
                    orig_cum += 1
                    if j == len(run) - 1:
                        kept.append(orig_cum)
                    else:
                        drops.append(mi)
                run.clear()
            for i in b.instructions:
                all_insts.append(i)
                nm = type(i).__name__
                if nm == "InstLdweights":
                    continue
                if nm == "InstMatmult" and upd_on(i):
                    run.append(i)
                    continue
                if upd_on(i):
                    flush()
                    orig_cum += 1
                    kept.append(orig_cum)
                    continue
                flush()
            flush()

    import bisect
    def remap(x):
        k = bisect.bisect_left(kept, x)
        return min(k + 1, len(kept))

    for i in drops:
        si = i.sync_info
        new_ups = [u for u in list(si.on_update)
                   if not (u.id == sem_id and u.update_mode == "sem-inc")]
        i.sync_info = bass_rust.SyncInfo(on_wait=list(si.on_wait),
                                         on_update=new_ups)
    for i in all_insts:
        si = i.sync_info
        if si is None:
            continue
        waits = list(si.on_wait)
        changed = False
        for w in waits:
            if w.id == sem_id and w.wait_mode == "sem-ge-imm":
                w.wait_value = remap(w.wait_value)
                changed = True
        if changed:
            i.sync_info = bass_rust.SyncInfo(on_wait=waits,
                                             on_update=list(si.on_update))


def build_program(T=2048, G=128, repeat=1):
    """Build the per-core Bass program (SPMD: same program, 8 data shards)."""
    nt = (F // T) * repeat
    nc = bass.Bass()
    yp = nc.declare_dram_parameter("yp", [P, F], F32, isOutput=False)
    yt = nc.declare_dram_parameter("yt", [P, F], F32, isOutput=False)
    st = nc.declare_dram_parameter("st", [P, F], I32, isOutput=False)
    iota_src = nc.declare_dram_parameter("iota_src", [P, L], F32, isOutput=False)
    out = nc.declare_dram_parameter("out", [L, 2 * H], F32, isOutput=True)

    with TileContext(nc) as tc:
        with (
            tc.tile_pool(name="consts", bufs=1) as pc,
            tc.tile_pool(name="inp", bufs=2) as pin,
            tc.tile_pool(name="mask", bufs=3) as pm,
            tc.tile_pool(name="acc", bufs=1, space="PSUM") as pp,
            tc.tile_pool(name="res", bufs=1) as pr,
        ):
            iof = pc.tile([P, L], F32)
            nc.sync.dma_start(out=iof[:], in_=iota_src[:])
            it = pc.tile([P, L], BF16)
            nc.vector.tensor_copy(it[:], iof[:])

            psum = pp.tile([L, 2 * H], F32)
            n_groups = nt * T
            gidx = 0
            for t in range(nt):
                tt = t % (F // T)
                sl = slice(tt * T, (tt + 1) * T)
                ypt = pin.tile([P, T], F32, tag="yp")
                nc.sync.dma_start(out=ypt[:], in_=yp[:, sl])
                ytt = pin.tile([P, T], F32, tag="yt")
                nc.sync.dma_start(out=ytt[:], in_=yt[:, sl])
                stt = pin.tile([P, T], I32, tag="st")
                nc.sync.dma_start(out=stt[:], in_=st[:, sl])

                diff = pin.tile([P, T], F32, tag="diff")
                nc.vector.tensor_sub(diff[:], ypt[:], ytt[:])
                v = pin.tile([P, T], BF16, tag="v")
                nc.scalar.activation(v[:], diff[:],
                                     mybir.ActivationFunctionType.Square)
                lo_i = pin.tile([P, T], I32, tag="lo_i")
                nc.vector.tensor_scalar(lo_i[:], stt[:], L - 1, None,
                                        mybir.AluOpType.bitwise_and)
                hi_i = pin.tile([P, T], I32, tag="hi_i")
                nc.vector.tensor_scalar(hi_i[:], stt[:], 5, None,
                                        mybir.AluOpType.arith_shift_right)
                lo = pin.tile([P, T], BF16, tag="lo")
                nc.vector.tensor_copy(lo[:], lo_i[:])
                hi = pin.tile([P, T], BF16, tag="hi")
                nc.vector.tensor_copy(hi[:], hi_i[:])

                for b in range(T // G):
                    gsl = slice(b * G, (b + 1) * G)
                    B3 = pm.tile([P, G, L], BF16, tag="B3")
                    AA = pm.tile([P, G, 2 * H], BF16, tag="AA")
                    # broadcast operand in slot 0 selects the faster DVE mode
                    itb = it[:].unsqueeze(1).broadcast_to([P, G, L])
                    lo_bc = lo[:, gsl].unsqueeze(2).broadcast_to([P, G, L])
                    nc.vector.tensor_tensor(B3[:], lo_bc, itb,
                                            mybir.AluOpType.is_equal)
                    ita = it[:].unsqueeze(1).broadcast_to([P, G, H])
                    hi_bc = hi[:, gsl].unsqueeze(2).broadcast_to([P, G, H])
                    nc.vector.tensor_tensor(AA[:, :, H:2 * H], hi_bc, ita,
                                            mybir.AluOpType.is_equal)
                    v_bc = v[:, gsl].unsqueeze(2).broadcast_to([P, G, H])
                    nc.vector.tensor_tensor(AA[:, :, 0:H], v_bc,
                                            AA[:, :, H:2 * H], mybir.AluOpType.mult)
                    for g in range(G):
                        ins = nc.tensor.matmul(psum[:], B3[:, g, :], AA[:, g, :],
                                               start=(gidx == 0),
                                               stop=(gidx == n_groups - 1))
                        ins.ins.is_weight_onezero = True
                        gidx += 1

            res = pr.tile([L, 2 * H], F32)
            nc.vector.tensor_copy(res[:], psum[:])
            nc.sync.dma_start(out=out[:], in_=res[:])
    _bulk_matmul_incs(nc)
    _split_multi_waits(nc)
    return nc


def make_consts():
    return {
        "iota_src": np.tile(np.arange(L, dtype=np.float32), (P, 1)),
    }


# --- PJRT runner (axon path): jitted shard_map over 8 cores with
# --- device-resident inputs, reusable across calls.

def _make_runner(nc, n_cores=N_CORES):
    import jax
    from jax.sharding import Mesh, PartitionSpec, NamedSharding
    from jax.experimental.shard_map import shard_map
    from concourse.bass2jax import (_bass_exec_p, install_neuronx_cc_hook,
                                    partition_id_tensor)

    install_neuronx_cc_hook()
    partition_name = nc.partition_id_tensor.name if nc.partition_id_tensor else None
    in_names, out_names, out_avals, zero_outs = [], [], [], []
    for alloc in nc.m.functions[0].allocations:
        if not isinstance(alloc, mybir.MemoryLocationSet):
            continue
        name = alloc.memorylocations[0].name
        if alloc.kind == "ExternalInput":
            if name != partition_name:
                in_names.append(name)
        elif alloc.kind == "ExternalOutput":
            out_names.append(name)
            shape = tuple(alloc.tensor_shape)
            dtype = mybir.dt.np(alloc.dtype)
            out_avals.append(jax.core.ShapedArray(shape, dtype))
            zero_outs.append(np.zeros(shape, dtype))
    n_params = len(in_names)
    n_outs = len(out_avals)
    all_in_names = list(in_names) + list(out_names)
    if partition_name is not None:
        all_in_names.append(partition_name)

    def _body(*args):
        operands = list(args)
        if partition_name is not None:
            operands.append(partition_id_tensor())
        return tuple(_bass_exec_p.bind(
            *operands,
            out_avals=tuple(out_avals),
            in_names=tuple(all_in_names),
            out_names=tuple(out_names),
            lowering_input_output_aliases=(),
            sim_require_finite=True,
            sim_require_nnan=True,
            nc=nc,
        ))

    devices = jax.devices()[:n_cores]
    mesh = Mesh(np.asarray(devices), ("core",))
    sharded = jax.jit(
        shard_map(_body, mesh=mesh,
                  in_specs=(PartitionSpec("core"),) * (n_params + n_outs),
                  out_specs=(PartitionSpec("core"),) * n_outs,
                  check_rep=False),
        keep_unused=True,
    )
    sh = NamedSharding(mesh, PartitionSpec("core"))

    class Runner:
        def put_inputs(self, in_maps):
            concat = [np.concatenate([np.asarray(m[n]) for m in in_maps], axis=0)
                      for n in in_names]
            self.dev_in = [jax.device_put(a, sh) for a in concat]
            self.zeros = [jax.device_put(
                np.zeros((n_cores * z.shape[0], *z.shape[1:]), z.dtype), sh)
                for z in zero_outs]
            jax.block_until_ready(self.dev_in)
            jax.block_until_ready(self.zeros)

        def run(self):
            outs = sharded(*self.dev_in, *self.zeros)
            jax.block_until_ready(outs)
            return outs

        def results(self, outs):
            return [{n: np.asarray(outs[i]).reshape(n_cores, *out_avals[i].shape)[c]
                     for i, n in enumerate(out_names)} for c in range(n_cores)]

    return Runner()


_RUNNER_CACHE = {}


def get_runner(repeat=1):
    if repeat not in _RUNNER_CACHE:
        _RUNNER_CACHE[repeat] = _make_runner(build_program(repeat=repeat))
    return _RUNNER_CACHE[repeat]


def shard_inputs(y_pred, y_true, stations):
    consts = make_consts()
    ypc = np.ascontiguousarray(y_pred, dtype=np.float32).reshape(N_CORES, P, F)
    ytc = np.ascontiguousarray(y_true, dtype=np.float32).reshape(N_CORES, P, F)
    stc = np.ascontiguousarray(stations, dtype=np.int32).reshape(N_CORES, P, F)
    return [{"yp": ypc[c], "yt": ytc[c], "st": stc[c], **consts}
            for c in range(N_CORES)]


def finish_host(partials, station_std):
    """Sum the 8 cores' partial [L, 2H] tiles and finish the loss on host
    (mirrors reference() in fp32)."""
    tot = np.sum(np.stack(partials, 0), axis=0, dtype=np.float32)
    sse = tot[:, 0:H].T.reshape(-1)         # index = hi*L + lo
    cnt = tot[:, H:2 * H].T.reshape(-1)
    mse = sse / np.maximum(cnt, np.float32(1.0))
    denom = (station_std.astype(np.float32) + np.float32(EPS)) ** 2
    present = cnt > 0
    per_station = np.where(present, mse / denom, np.float32(0.0))
    n_present = np.maximum(np.sum(present.astype(np.float32)), np.float32(1.0))
    return (np.sum(per_station) / n_present).astype(np.float32)


def kernel(y_pred, y_true, stations, station_std):
    runner = get_runner()
    runner.put_inputs(shard_inputs(y_pred, y_true, stations))
    outs = runner.run()
    res = runner.results(outs)
    partials = [res[c]["out"] for c in range(N_CORES)]
    loss = finish_host(partials, np.asarray(station_std))
    return np.asarray(loss, dtype=np.float32)
